# revision 26
# baseline (speedup 1.0000x reference)
"""TRN2 Bass kernel for nn_OFTLinear (forward).

Math: the whole OFT chain is linear, so
    out = x @ W_eff + b_eff
with
    W_eff = P_in . BD(R_right) . W^T . BD(R_left) . P_out      [2048 x 2048]
    b_eff = (BD(R_left)^T b)[inv_perm_out]
where R = Cayley-Neumann(skew(oft)) per 32x32 block, BD() is block-diagonal,
and P_in/P_out are the input/output feature permutations.

Device pipeline (replicated on all 8 cores; x sharded along tokens). The
whole W_eff build + GEMM runs in bf16 (fp32 PSUM accumulation): rel-err
budget is 2e-2 and bf16 lands ~4e-3. The Cayley series itself also runs in
bf16 -- its tiles are cast to bf16 for the matmuls anyway, so fp32
intermediates only added PE time (fp32 matmuls are 4x slower).

  Q:  Q_flat = vec^T @ E (E: host-built one-hot skew-scatter matrix),
      4-deep load buffers so the DMA/matmul ping-pong doesn't serialize.
  C:  BD4 tiles of Q (4 blocks per 128x128 tile) -> Cayley powers on PE ->
      R_left tiles (g<16) and R_right^T = R(-Q) tiles (g>=16), all bf16.
  B:  b_rot = BD_L^T b via 16 matvecs; stored as an EXTRA COLUMN (col 2048)
      of H in DRAM via one strided DMA, so the same row-gather that permutes
      W_eff also permutes the bias -- no indirect scatter chain.
  H:  H = BD_L^T @ W on PE, streamed to DRAM as [2048, 2176] bf16 rows
      (cols 0..2047 = H, col 2048 = bias, rest pad). W tiles are prefetched
      8 deep so the loads run during the Cayley phase.
  T+G: 8 dma_gather(transpose=True) ops (256 out-cols each) deliver
      H2T = gathered(H)^T into SBUF; each gather is immediately followed by
      its 16 G matmuls (G = BD(R_right) @ H2T chunk) into W_eff slices, so
      gathers and G pipeline per-gc instead of forming one serial wall.
  GEMM: W_eff k-tiles live in SBUF; out = xT.T@W_eff + bias. x supertiles
      are software-pipelined (prefetch depth 2) on the SP DMA queue, which
      is otherwise idle until output stores start.

Constants (identity, gather indices, bias layout) are built OUTSIDE the
n_reps hardware loop so they don't re-execute per iteration.

Host does layout-only work: shard x along tokens, transpose + bf16-cast each
shard (fp32 DMA transpose is unsupported on this stack), concat + bf16-cast
oft_L/oft_R, and build integer index/one-hot constants from the
permutation/index buffers.

n_reps > 1 wraps the computation in a tc.For_i hardware loop so one
dispatch executes the full kernel n_reps times back-to-back: per-iteration
HW time can then be measured above the host-side dispatch latency (which
dwarfs a single execution in this environment).
"""

import numpy as np

IN_F = 2048
OUT_F = 2048
BS = 32
N_ELEM = BS * (BS - 1) // 2  # 496
N_BLOCKS = 128  # 64 left + 64 right
N_CORES = 8
TOKENS = 4 * 8192
TOKPC = TOKENS // N_CORES  # 4096
KB = IN_F // 128  # 16 k-blocks
NB = OUT_F // 128  # 16 n-blocks
HCOL = IN_F + 128  # H DRAM row: 2048 cols + bias col + 127 pad (2176)

_CACHE = {}


def _build(tokpc, n_reps=1, debug=False):
    import concourse.bass as bass
    import concourse.bacc as bacc
    import concourse.mybir as mybir
    import concourse.tile as tile
    from concourse.masks import make_identity

    dt = mybir.dt
    BF = dt.bfloat16

    SUP = 256  # token super-tile
    n_sup = tokpc // SUP
    MT = SUP // 128  # m-tiles per super

    nc = bacc.Bacc(None, target_bir_lowering=False, debug=False,
                   enable_asserts=False, num_devices=1, num_swdge_queues=4)

    xt_in = nc.dram_tensor("xt", [IN_F, tokpc], BF, kind="ExternalInput").ap()
    w_in = nc.dram_tensor("w", [OUT_F, IN_F], BF, kind="ExternalInput").ap()
    b_in = nc.dram_tensor("b", [OUT_F, 1], BF, kind="ExternalInput").ap()
    # oft pre-transposed on host (layout-only): [N_ELEM, N_BLOCKS]
    oft_in = nc.dram_tensor("oft", [N_ELEM, N_BLOCKS], BF, kind="ExternalInput").ap()
    emat_in = nc.dram_tensor("emat", [N_ELEM, BS * BS], BF, kind="ExternalInput").ap()
    # inverse out-perm as wrapped int16 gather indices: [128, 8*16]
    gout_in = nc.dram_tensor("gout", [128, 8 * NB], dt.int16, kind="ExternalInput").ap()
    out_d = nc.dram_tensor("out", [tokpc, OUT_F], dt.float32, kind="ExternalOutput").ap()

    qflat_d = nc.dram_tensor("qflat_d", [N_BLOCKS, BS, BS], BF).ap()
    dbg_kw = {"kind": "ExternalOutput"} if debug else {}
    hnat_d = nc.dram_tensor("hnat_d", [OUT_F, HCOL], BF, **dbg_kw).ap()
    if debug:
        wdump_d = nc.dram_tensor("wdump", [IN_F, OUT_F], BF,
                                 kind="ExternalOutput").ap()
        bdump_d = nc.dram_tensor("bdump", [128, OUT_F], dt.float32,
                                 kind="ExternalOutput").ap()

    def _emit_body(tc, C):
        """One kernel execution. C holds the preloaded constants."""
        rq_by_q = {}

        def r_tile(g):
            return rq_by_q[g // 4][:, g % 4, :]

        xts_tiles = {}

        def make_xts_loader(sbg):
            xt_view = xt_in[:].rearrange("(k p) t -> p k t", p=128)

            def load_xts(s):
                t = sbg.tile([128, KB, SUP], BF, tag="xts")
                nc.sync.dma_start(t[:], xt_view[:, :, s * SUP:(s + 1) * SUP])
                xts_tiles[s] = t
            return load_xts

        # ---------------- Phase Q: Q_flat = vec^T @ E ----------------
        with tc.tile_pool(name="bdqp", bufs=1) as bdqp:
            bdq_all = bdqp.tile([128, 32, 128], BF)
            nc.vector.memset(bdq_all[:], 0.0)
            with tc.tile_pool(name="sbq", bufs=4) as sbq, \
                 tc.tile_pool(name="psq", bufs=1, space="PSUM") as psq:
                qps = psq.tile([128, BS * BS], dt.float32)
                CH = 124
                for c in range(4):
                    lo = c * CH
                    sz = min(CH, N_ELEM - lo)
                    vt = sbq.tile([CH, 128], BF, tag="vt")
                    nc.scalar.dma_start(vt[:sz, :], oft_in[lo:lo + sz, :])
                    et = sbq.tile([CH, BS * BS], BF, tag="et")
                    nc.scalar.dma_start(et[:sz, :], emat_in[lo:lo + sz, :])
                    for nh in range(2):
                        nc.tensor.matmul(out=qps[:, nh * 512:(nh + 1) * 512],
                                         lhsT=vt[:sz, :],
                                         rhs=et[:sz, nh * 512:(nh + 1) * 512],
                                         start=(c == 0), stop=(c == 3))
                qsb = sbq.tile([128, BS * BS], BF)
                nc.vector.tensor_copy(out=qsb[:], in_=qps[:])
                nc.scalar.dma_start(qflat_d[:].rearrange("p a b -> p (a b)"),
                                     qsb[:])

            # BD4 layout: quad q slot s holds blocks 4*(4q+s)..4*(4q+s)+3
            qview = qflat_d[:].rearrange("(g four) i j -> four i g j", four=4)
            for r in range(4):
                nc.scalar.dma_start(
                    bdq_all[r * BS:(r + 1) * BS, :, r * BS:(r + 1) * BS],
                    qview[r])

            # ---------------- Phase C: Cayley powers (bf16) ----------------
            def cayley_quad(q, sbc, psc):
                """Generator: one DVE<->PE pipeline step per yield so quads
                interleave their latency chains. Sign-flipped intermediates
                fold the Neumann x2 scaling into the matmuls.
                   R(+Q) = I + (2Q^4 + 2Q^2) + (2Q + 2Q^3)   (q < 4)
                   R(-Q) = I + (2Q^4 + 2Q^2) - (2Q + 2Q^3)   (q >= 4)
                """
                ev = nc.vector
                sub = mybir.AluOpType.subtract
                bdq4 = bdq_all[:, 4 * q:4 * q + 4, :]
                s2q = sbc.tile([128, 4, 128], BF, tag="s2q")
                ev.tensor_scalar_mul(out=s2q[:], in0=bdq4, scalar1=2.0)
                yield
                m2p2ps = psc.tile([128, 4, 128], dt.float32, tag="p2ps")
                for gg in range(4):
                    nc.tensor.matmul(out=m2p2ps[:, gg, :], lhsT=bdq4[:, gg, :],
                                     rhs=s2q[:, gg, :], start=True, stop=True)
                m2p2 = sbc.tile([128, 4, 128], BF, tag="p2")
                nc.scalar.copy(out=m2p2[:], in_=m2p2ps[:])
                yield
                m2p3ps = psc.tile([128, 4, 128], dt.float32, tag="p3ps")
                for gg in range(4):
                    nc.tensor.matmul(out=m2p3ps[:, gg, :], lhsT=m2p2[:, gg, :],
                                     rhs=bdq4[:, gg, :], start=True, stop=True)
                m2p3 = sbc.tile([128, 4, 128], BF, tag="p3")
                nc.scalar.copy(out=m2p3[:], in_=m2p3ps[:])
                yield
                p4ps = psc.tile([128, 4, 128], dt.float32, tag="p2ps")
                for gg in range(4):
                    nc.tensor.matmul(out=p4ps[:, gg, :], lhsT=m2p3[:, gg, :],
                                     rhs=bdq4[:, gg, :], start=True, stop=True)
                t1 = sbc.tile([128, 4, 128], BF, tag="t1")
                ev.tensor_tensor(out=t1[:], in0=p4ps[:], in1=m2p2[:], op=sub)
                yield
                t2 = sbc.tile([128, 4, 128], BF, tag="t2")
                ev.tensor_tensor(out=t2[:], in0=s2q[:], in1=m2p3[:], op=sub)
                t3 = sbc.tile([128, 4, 128], BF, tag="t3")
                op = mybir.AluOpType.add if q < 4 else sub
                ev.tensor_tensor(out=t3[:], in0=t1[:], in1=t2[:], op=op)
                yield
                rq = C["rpool"].tile([128, 4, 128], BF, tag="rq",
                                     name=f"rq_{q}")
                ev.tensor_add(out=rq[:], in0=t3[:], in1=C["identq"][:])
                rq_by_q[q] = rq

            def drive(gens):
                gens = list(gens)
                while gens:
                    g = gens.pop(0)
                    try:
                        next(g)
                        gens.append(g)
                    except StopIteration:
                        pass

            with tc.tile_pool(name="sbc", bufs=2) as sbc, \
                 tc.tile_pool(name="psc", bufs=4, space="PSUM") as psc:
                drive([cayley_quad(q, sbc, psc) for q in (0, 1, 2, 3)])

            # Phase B: b_rot = BD_L^T b -> bias column (2048) of hnat_d
            # via one strided DMA store (bf16 R_left tiles; bias is tiny).
            with tc.tile_pool(name="sbb", bufs=1) as sbb, \
                 tc.tile_pool(name="psb", bufs=1, space="PSUM") as psb:
                brotps = psb.tile([128, NB], dt.float32)
                for g in range(NB):
                    nc.tensor.matmul(out=brotps[:, g:g + 1],
                                     lhsT=rq_by_q[g // 4][:, g % 4, :],
                                     rhs=C["b_sb"][:, g:g + 1],
                                     start=True, stop=True)
                brot = sbb.tile([128, NB], BF)
                nc.scalar.copy(out=brot[:], in_=brotps[:])
                nc.sync.dma_start(
                    hnat_d[:].rearrange("(g p) c -> p g c", p=128)[
                        :, :, IN_F:IN_F + 1].rearrange("p g one -> p (g one)"),
                    brot[:])

            # ------- Phase H: H = BD_L^T @ W -> DRAM rows, bf16 -------
            # Needs only quads 0-3; quads 4-7 (for G) are driven right
            # after so their DVE/PE latency chains hide under H's
            # DMA-gated execution.
            with tc.tile_pool(name="wpool", bufs=16) as wpool, \
                 tc.tile_pool(name="hstp", bufs=6) as hstp, \
                 tc.tile_pool(name="psh", bufs=2, space="PSUM") as psh:
                for g in range(NB):
                    wt = wpool.tile([128, IN_F], BF, tag="wt")
                    nc.scalar.dma_start(wt[:], w_in[g * 128:(g + 1) * 128, :])
                    hps = psh.tile([128, IN_F], dt.float32, tag="hps")
                    for n in range(IN_F // 512):
                        nc.tensor.matmul(out=hps[:, n * 512:(n + 1) * 512],
                                         lhsT=r_tile(g),
                                         rhs=wt[:, n * 512:(n + 1) * 512],
                                         start=True, stop=True)
                    hsb = hstp.tile([128, IN_F], BF, tag="hsb")
                    # Pool/GPSIMD cannot read PSUM on HW: DVE/Act only.
                    if g % 2 == 0:
                        nc.vector.tensor_copy(out=hsb[:], in_=hps[:])
                    else:
                        nc.scalar.copy(out=hsb[:], in_=hps[:])
                    nc.sync.dma_start(hnat_d[g * 128:(g + 1) * 128, :IN_F],
                                      hsb[:])

            with tc.tile_pool(name="sbc2", bufs=2) as sbc2, \
                 tc.tile_pool(name="psc2", bufs=4, space="PSUM") as psc2:
                drive([cayley_quad(q, sbc2, psc2) for q in (4, 5, 6, 7)])

        # --- Phase T+G: transpose-gather rows of H(+bias) by inv_perm_out,
        #     fused per-gc with G = BD(R_right) @ H2T chunk matmuls.
        with tc.tile_pool(name="sbg", bufs=3) as sbg, \
             tc.tile_pool(name="biasp", bufs=1) as biasp:
            load_xts = make_xts_loader(sbg)
            load_xts(0)
            load_xts(1)
            bias_sb = biasp.tile([128, OUT_F], dt.float32)

            with tc.tile_pool(name="h2tp", bufs=2) as h2tp, \
                 tc.tile_pool(name="psg", bufs=1, space="PSUM") as psg:
                for gc in range(NB // 4):
                    h2t = h2tp.tile([128, KB + 1, 512], BF, tag="h2t")
                    nc.gpsimd.dma_gather(
                        out_ap=h2t[:],
                        in_ap=hnat_d[:],
                        idxs_ap=C["gidx"][:, gc * 32:(gc + 1) * 32],
                        num_idxs=512, num_idxs_reg=512, elem_size=HCOL,
                        transpose=True, queue_num=1 + gc % 2)
                    for sub in range(2):
                        ss = slice(sub * 256, (sub + 1) * 256)
                        for pair in range(KB // 2):
                            pr = psg.tile([128, 2, 256], dt.float32,
                                          tag=f"gp{pair}")
                            for half in range(2):
                                i = pair * 2 + half
                                nc.tensor.matmul(out=pr[:, half, :],
                                                 lhsT=r_tile(16 + i),
                                                 rhs=h2t[:, i, ss],
                                                 start=True, stop=True)
                            col0 = gc * 512 + sub * 256
                            dst = C["weff"][:, 2 * pair:2 * pair + 2,
                                            col0:col0 + 256]
                            if pair % 2 == 0:
                                nc.vector.tensor_copy(out=dst, in_=pr[:])
                            else:
                                nc.scalar.copy(out=dst, in_=pr[:])
                    # permuted bias row for this gc (partition 0, chunk KB)
                    nc.vector.tensor_copy(
                        out=C["b2row"][0:1, gc * 512:(gc + 1) * 512],
                        in_=h2t[0:1, KB, :])

            # bias broadcast across partitions via K=1 ones-matmul
            with tc.tile_pool(name="psbias", bufs=1, space="PSUM") as psbias:
                bbps = psbias.tile([128, OUT_F], dt.float32)
                for n in range(OUT_F // 512):
                    nc.tensor.matmul(out=bbps[:, n * 512:(n + 1) * 512],
                                     lhsT=C["onesb"][:1, :],
                                     rhs=C["b2row"][:1, n * 512:(n + 1) * 512],
                                     start=True, stop=True)
                nc.vector.tensor_copy(out=bias_sb[:], in_=bbps[:])

            if debug:
                nc.sync.dma_start(
                    wdump_d[:].rearrange("(k p) o -> p (k o)", p=128),
                    C["weff"][:].rearrange("p k o -> p (k o)"))
                nc.sync.dma_start(bdump_d[:], bias_sb[:])

            # ---------------- main GEMM ----------------
            with tc.tile_pool(name="osbp", bufs=2) as osbp, \
                 tc.tile_pool(name="psgm", bufs=2, space="PSUM") as psgm:
                for s in range(n_sup):
                    if s + 2 < n_sup:
                        load_xts(s + 2)
                    xts = xts_tiles.pop(s)
                    for mt in range(MT):
                        gps = psgm.tile([128, OUT_F], dt.float32, tag="gemmps")
                        for k in range(KB):
                            for n in range(OUT_F // 512):
                                nc.tensor.matmul(
                                    out=gps[:, n * 512:(n + 1) * 512],
                                    lhsT=xts[:, k, mt * 128:(mt + 1) * 128],
                                    rhs=C["weff"][:, k, n * 512:(n + 1) * 512],
                                    start=(k == 0), stop=(k == KB - 1))
                        osb = osbp.tile([128, OUT_F], dt.float32, tag="osb")
                        row0 = s * SUP + mt * 128
                        last = (s == n_sup - 1) and (mt == MT - 1)
                        if not last:
                            nc.vector.tensor_add(out=osb[:], in0=gps[:],
                                                 in1=bias_sb[:])
                            nc.sync.dma_start(out_d[row0:row0 + 128, :],
                                              osb[:])
                        else:
                            # chunk the final tile so add+store pipeline and
                            # the drain tail shrinks
                            for n in range(4):
                                sl = slice(n * 512, (n + 1) * 512)
                                nc.vector.tensor_add(out=osb[:, sl],
                                                     in0=gps[:, sl],
                                                     in1=bias_sb[:, sl])
                                nc.sync.dma_start(out_d[row0:row0 + 128, sl],
                                                  osb[:, sl])

    with tile.TileContext(nc) as tc:
        # Constants + long-lived tiles, emitted once (outside the HW loop).
        with tc.tile_pool(name="const", bufs=1) as const, \
             tc.tile_pool(name="wfp", bufs=1) as wfp, \
             tc.tile_pool(name="b2rp", bufs=1) as b2rp, \
             tc.tile_pool(name="rpool", bufs=8) as rpool:
            ident = const.tile([128, 128], dt.float32)
            make_identity(nc, ident)
            identq = const.tile([128, 4, 128], BF)
            for gg in range(4):
                nc.vector.tensor_copy(out=identq[:, gg, :], in_=ident[:])
            gidx = const.tile([128, 8 * NB], dt.int16)
            nc.sync.dma_start(gidx[:], gout_in[:])
            b_sb = const.tile([128, NB], BF)
            nc.sync.dma_start(
                b_sb[:], b_in[:].rearrange("(g p) one -> p (g one)", p=128))
            onesb = const.tile([1, 128], BF)
            nc.vector.memset(onesb[:], 1.0)
            C = {
                "identq": identq, "gidx": gidx, "b_sb": b_sb, "onesb": onesb,
                "weff": wfp.tile([128, KB, OUT_F], BF, name="weff_all"),
                "b2row": b2rp.tile([1, OUT_F], BF, name="b2row"),
                "rpool": rpool,
            }
            if n_reps > 1:
                # Whole-kernel hardware loop: executes the full computation
                # n_reps times per dispatch so per-iteration HW time can be
                # measured above the host-side dispatch latency.
                with tc.For_i(0, n_reps):
                    _emit_body(tc, C)
            else:
                _emit_body(tc, C)

    nc.compile()
    return nc


def _wrap_idx16(idx):
    """Pack N gather indices into dma_gather's wrapped layout: index j at
    [j % 16, j // 16], replicated across the 8 Q7 cores -> [128, N//16]."""
    n = len(idx)
    arr = np.zeros((16, n // 16), np.int16)
    j = np.arange(n)
    arr[j % 16, j // 16] = idx.astype(np.int16)
    return np.tile(arr, (8, 1))


def _host_prep(inputs):
    from ml_dtypes import bfloat16
    rows = np.asarray(inputs["rows"]).astype(np.int64)
    cols = np.asarray(inputs["cols"]).astype(np.int64)
    emat = np.zeros((N_ELEM, BS * BS), dtype=np.float32)
    e_idx = np.arange(N_ELEM)
    emat[e_idx, rows * BS + cols] = 1.0
    emat[e_idx, cols * BS + rows] = -1.0
    emat = emat.astype(bfloat16)
    oft = np.ascontiguousarray(
        np.concatenate([np.asarray(inputs["oft_L"], dtype=np.float32),
                        np.asarray(inputs["oft_R"], dtype=np.float32)],
                       axis=0).T).astype(bfloat16)  # host-transposed [496, 128]
    inv_pout = np.asarray(inputs["inv_perm_out"]).astype(np.int64)
    gout = np.concatenate([_wrap_idx16(inv_pout[gc * 256:(gc + 1) * 256])
                           for gc in range(NB // 2)], axis=1)
    w = np.ascontiguousarray(np.asarray(inputs["W"], dtype=np.float32)).astype(bfloat16)
    b = np.asarray(inputs["b"], dtype=np.float32).reshape(OUT_F, 1).astype(bfloat16)
    return emat, oft, gout, w, b


def _in_map(inputs):
    emat, oft, gout, w, b = _host_prep(inputs)
    return {"w": w, "b": b, "oft": oft, "emat": emat, "gout": gout}


def kernel(**inputs):
    from concourse.bass_utils import run_bass_kernel_spmd
    from ml_dtypes import bfloat16

    key = ("full", TOKPC)
    if key not in _CACHE:
        _CACHE[key] = _build(TOKPC)
    nc = _CACHE[key]

    x = np.asarray(inputs["x"], dtype=np.float32).reshape(TOKENS, IN_F)
    perm_in = np.asarray(inputs["perm_in"]).astype(np.int64)
    base = _in_map(inputs)
    in_maps = []
    for c in range(N_CORES):
        m = dict(base)
        # host-side layout: transpose the shard AND apply the input feature
        # permutation as a row gather (x @ P_in == P_in-rows of x^T)
        m["xt"] = np.ascontiguousarray(
            x[c * TOKPC:(c + 1) * TOKPC].T[perm_in]).astype(bfloat16)
        in_maps.append(m)

    res = run_bass_kernel_spmd(nc, in_maps, core_ids=list(range(N_CORES)))
    out = np.concatenate([res.results[c]["out"] for c in range(N_CORES)], axis=0)
    return out.reshape(4, 8192, OUT_F)


# revision 29
# speedup vs baseline: 1.0084x; 1.0084x over previous
"""TRN2 Bass kernel for nn_OFTLinear (forward).

Math: the whole OFT chain is linear, so
    out = x @ W_eff + b_eff
with
    W_eff = P_in . BD(R_right) . W^T . BD(R_left) . P_out      [2048 x 2048]
    b_eff = (BD(R_left)^T b)[inv_perm_out]
where R = Cayley-Neumann(skew(oft)) per 32x32 block, BD() is block-diagonal,
and P_in/P_out are the input/output feature permutations.

Device pipeline (replicated on all 8 cores; x sharded along tokens). The
whole W_eff build + GEMM runs in bf16 (fp32 PSUM accumulation): rel-err
budget is 2e-2 and bf16 lands ~4e-3. The Cayley series itself also runs in
bf16 -- its tiles are cast to bf16 for the matmuls anyway, so fp32
intermediates only added PE time (fp32 matmuls are 4x slower).

  Q:  Q_flat = vec^T @ E (E: host-built one-hot skew-scatter matrix),
      4-deep load buffers so the DMA/matmul ping-pong doesn't serialize.
  C:  BD4 tiles of Q (4 blocks per 128x128 tile) -> Cayley powers on PE ->
      R_left tiles (g<16) and R_right^T = R(-Q) tiles (g>=16), all bf16.
  B:  b_rot = BD_L^T b via 16 matvecs; stored as an EXTRA COLUMN (col 2048)
      of H in DRAM via one strided DMA, so the same row-gather that permutes
      W_eff also permutes the bias -- no indirect scatter chain.
  H:  H = BD_L^T @ W on PE, streamed to DRAM as [2048, 2176] bf16 rows
      (cols 0..2047 = H, col 2048 = bias, rest pad). W tiles are prefetched
      8 deep so the loads run during the Cayley phase.
  T+G: 8 dma_gather(transpose=True) ops (256 out-cols each) deliver
      H2T = gathered(H)^T into SBUF; each gather is immediately followed by
      its 16 G matmuls (G = BD(R_right) @ H2T chunk) into W_eff slices, so
      gathers and G pipeline per-gc instead of forming one serial wall.
  GEMM: W_eff k-tiles live in SBUF; out = xT.T@W_eff + bias. x supertiles
      are software-pipelined (prefetch depth 2) on the SP DMA queue, which
      is otherwise idle until output stores start.

Constants (identity, gather indices, bias layout) are built OUTSIDE the
n_reps hardware loop so they don't re-execute per iteration.

Host does layout-only work: shard x along tokens, transpose + bf16-cast each
shard (fp32 DMA transpose is unsupported on this stack), concat + bf16-cast
oft_L/oft_R, and build integer index/one-hot constants from the
permutation/index buffers.

n_reps > 1 wraps the computation in a tc.For_i hardware loop so one
dispatch executes the full kernel n_reps times back-to-back: per-iteration
HW time can then be measured above the host-side dispatch latency (which
dwarfs a single execution in this environment).
"""

import numpy as np

IN_F = 2048
OUT_F = 2048
BS = 32
N_ELEM = BS * (BS - 1) // 2  # 496
N_BLOCKS = 128  # 64 left + 64 right
N_CORES = 8
TOKENS = 4 * 8192
TOKPC = TOKENS // N_CORES  # 4096
KB = IN_F // 128  # 16 k-blocks
NB = OUT_F // 128  # 16 n-blocks
HCOL = IN_F + 128  # H DRAM row: 2048 cols + bias col + 127 pad (2176)

_CACHE = {}


def _build(tokpc, n_reps=1, debug=False):
    import concourse.bass as bass
    import concourse.bacc as bacc
    import concourse.mybir as mybir
    import concourse.tile as tile
    from concourse.masks import make_identity

    dt = mybir.dt
    BF = dt.bfloat16

    SUP = 256  # token super-tile
    n_sup = tokpc // SUP
    MT = SUP // 128  # m-tiles per super

    nc = bacc.Bacc(None, target_bir_lowering=False, debug=False,
                   enable_asserts=False, num_devices=1, num_swdge_queues=4)

    xt_in = nc.dram_tensor("xt", [IN_F, tokpc], BF, kind="ExternalInput").ap()
    w_in = nc.dram_tensor("w", [OUT_F, IN_F], BF, kind="ExternalInput").ap()
    b_in = nc.dram_tensor("b", [OUT_F, 1], BF, kind="ExternalInput").ap()
    # oft pre-transposed on host (layout-only): [N_ELEM, N_BLOCKS]
    oft_in = nc.dram_tensor("oft", [N_ELEM, N_BLOCKS], BF, kind="ExternalInput").ap()
    emat_in = nc.dram_tensor("emat", [N_ELEM, BS * BS], BF, kind="ExternalInput").ap()
    # inverse out-perm as wrapped int16 gather indices: [128, 8*16]
    gout_in = nc.dram_tensor("gout", [128, 8 * NB], dt.int16, kind="ExternalInput").ap()
    out_d = nc.dram_tensor("out", [tokpc, OUT_F], dt.float32, kind="ExternalOutput").ap()

    qflat_d = nc.dram_tensor("qflat_d", [N_BLOCKS, BS, BS], BF).ap()
    dbg_kw = {"kind": "ExternalOutput"} if debug else {}
    hnat_d = nc.dram_tensor("hnat_d", [OUT_F, HCOL], BF, **dbg_kw).ap()
    if debug:
        wdump_d = nc.dram_tensor("wdump", [IN_F, OUT_F], BF,
                                 kind="ExternalOutput").ap()
        bdump_d = nc.dram_tensor("bdump", [128, OUT_F], dt.float32,
                                 kind="ExternalOutput").ap()

    def _emit_body(tc, C):
        """One kernel execution. C holds the preloaded constants."""
        rq_by_q = {}

        def r_tile(g):
            return rq_by_q[g // 4][:, g % 4, :]

        xts_tiles = {}

        def make_xts_loader(sbg):
            xt_view = xt_in[:].rearrange("(k p) t -> p k t", p=128)

            def load_xts(s):
                t = sbg.tile([128, KB, SUP], BF, tag="xts")
                nc.sync.dma_start(t[:], xt_view[:, :, s * SUP:(s + 1) * SUP])
                xts_tiles[s] = t
            return load_xts

        # ---------------- Phase Q: Q_flat = vec^T @ E ----------------
        with tc.tile_pool(name="bdqp", bufs=1) as bdqp, \
             tc.tile_pool(name="wpool", bufs=16) as wpool:
            bdq_all = bdqp.tile([128, 32, 128], BF)
            nc.vector.memset(bdq_all[:], 0.0)
            with tc.tile_pool(name="sbq", bufs=4) as sbq, \
                 tc.tile_pool(name="psq", bufs=1, space="PSUM") as psq:
                qps = psq.tile([128, BS * BS], dt.float32)
                CH = 124
                for c in range(4):
                    lo = c * CH
                    sz = min(CH, N_ELEM - lo)
                    vt = sbq.tile([CH, 128], BF, tag="vt")
                    nc.scalar.dma_start(vt[:sz, :], oft_in[lo:lo + sz, :])
                    et = sbq.tile([CH, BS * BS], BF, tag="et")
                    nc.scalar.dma_start(et[:sz, :], emat_in[lo:lo + sz, :])
                    for nh in range(2):
                        nc.tensor.matmul(out=qps[:, nh * 512:(nh + 1) * 512],
                                         lhsT=vt[:sz, :],
                                         rhs=et[:sz, nh * 512:(nh + 1) * 512],
                                         start=(c == 0), stop=(c == 3))
                # W loads up front: the scalar engine reaches these while the
                # PREVIOUS iteration's GEMM still owns the PE, so all of W is
                # in SBUF before this iteration's H phase starts.
                wts = []
                for g in range(NB):
                    wt = wpool.tile([128, IN_F], BF, tag="wt", name=f"wt_{g}")
                    nc.scalar.dma_start(wt[:],
                                        w_in[g * 128:(g + 1) * 128, :])
                    wts.append(wt)
                qsb = sbq.tile([128, BS * BS], BF)
                nc.vector.tensor_copy(out=qsb[:], in_=qps[:])
                nc.scalar.dma_start(qflat_d[:].rearrange("p a b -> p (a b)"),
                                     qsb[:])

            # BD4 layout: quad q slot s holds blocks 4*(4q+s)..4*(4q+s)+3
            qview = qflat_d[:].rearrange("(g four) i j -> four i g j", four=4)
            for r in range(4):
                nc.scalar.dma_start(
                    bdq_all[r * BS:(r + 1) * BS, :, r * BS:(r + 1) * BS],
                    qview[r])

            # ---------------- Phase C: Cayley powers (bf16) ----------------
            def cayley_quad(q, sbc, psc):
                """Generator: one DVE<->PE pipeline step per yield so quads
                interleave their latency chains. Sign-flipped intermediates
                fold the Neumann x2 scaling into the matmuls.
                   R(+Q) = I + (2Q^4 + 2Q^2) + (2Q + 2Q^3)   (q < 4)
                   R(-Q) = I + (2Q^4 + 2Q^2) - (2Q + 2Q^3)   (q >= 4)
                """
                ev = nc.vector
                sub = mybir.AluOpType.subtract
                bdq4 = bdq_all[:, 4 * q:4 * q + 4, :]
                s2q = sbc.tile([128, 4, 128], BF, tag="s2q")
                ev.tensor_scalar_mul(out=s2q[:], in0=bdq4, scalar1=2.0)
                yield
                m2p2ps = psc.tile([128, 4, 128], dt.float32, tag="p2ps")
                for gg in range(4):
                    nc.tensor.matmul(out=m2p2ps[:, gg, :], lhsT=bdq4[:, gg, :],
                                     rhs=s2q[:, gg, :], start=True, stop=True)
                m2p2 = sbc.tile([128, 4, 128], BF, tag="p2")
                nc.scalar.copy(out=m2p2[:], in_=m2p2ps[:])
                yield
                m2p3ps = psc.tile([128, 4, 128], dt.float32, tag="p3ps")
                for gg in range(4):
                    nc.tensor.matmul(out=m2p3ps[:, gg, :], lhsT=m2p2[:, gg, :],
                                     rhs=bdq4[:, gg, :], start=True, stop=True)
                m2p3 = sbc.tile([128, 4, 128], BF, tag="p3")
                nc.scalar.copy(out=m2p3[:], in_=m2p3ps[:])
                yield
                p4ps = psc.tile([128, 4, 128], dt.float32, tag="p2ps")
                for gg in range(4):
                    nc.tensor.matmul(out=p4ps[:, gg, :], lhsT=m2p3[:, gg, :],
                                     rhs=bdq4[:, gg, :], start=True, stop=True)
                t1 = sbc.tile([128, 4, 128], BF, tag="t1")
                ev.tensor_tensor(out=t1[:], in0=p4ps[:], in1=m2p2[:], op=sub)
                yield
                t2 = sbc.tile([128, 4, 128], BF, tag="t2")
                ev.tensor_tensor(out=t2[:], in0=s2q[:], in1=m2p3[:], op=sub)
                t3 = sbc.tile([128, 4, 128], BF, tag="t3")
                op = mybir.AluOpType.add if q < 4 else sub
                ev.tensor_tensor(out=t3[:], in0=t1[:], in1=t2[:], op=op)
                yield
                rq = C["rpool"].tile([128, 4, 128], BF, tag="rq",
                                     name=f"rq_{q}")
                ev.tensor_add(out=rq[:], in0=t3[:], in1=C["identq"][:])
                rq_by_q[q] = rq

            def drive(gens):
                gens = list(gens)
                while gens:
                    g = gens.pop(0)
                    try:
                        next(g)
                        gens.append(g)
                    except StopIteration:
                        pass

            with tc.tile_pool(name="sbc", bufs=2) as sbc, \
                 tc.tile_pool(name="psc", bufs=4, space="PSUM") as psc:
                drive([cayley_quad(q, sbc, psc) for q in (0, 1, 2, 3)])

            # Phase B: b_rot = BD_L^T b -> bias column (2048) of hnat_d
            # via one strided DMA store (bf16 R_left tiles; bias is tiny).
            with tc.tile_pool(name="sbb", bufs=1) as sbb, \
                 tc.tile_pool(name="psb", bufs=1, space="PSUM") as psb:
                brotps = psb.tile([128, NB], dt.float32)
                for g in range(NB):
                    nc.tensor.matmul(out=brotps[:, g:g + 1],
                                     lhsT=rq_by_q[g // 4][:, g % 4, :],
                                     rhs=C["b_sb"][:, g:g + 1],
                                     start=True, stop=True)
                brot = sbb.tile([128, NB], BF)
                nc.scalar.copy(out=brot[:], in_=brotps[:])
                nc.sync.dma_start(
                    hnat_d[:].rearrange("(g p) c -> p g c", p=128)[
                        :, :, IN_F:IN_F + 1].rearrange("p g one -> p (g one)"),
                    brot[:])

            # ------- Phase H: H = BD_L^T @ W -> DRAM rows, bf16 -------
            # Needs only quads 0-3; quads 4-7 (for G) are driven right
            # after so their DVE/PE latency chains hide under H's
            # DMA-gated execution.
            with tc.tile_pool(name="hstp", bufs=6) as hstp, \
                 tc.tile_pool(name="psh", bufs=2, space="PSUM") as psh:
                for g in range(NB):
                    hps = psh.tile([128, IN_F], dt.float32, tag="hps")
                    for n in range(IN_F // 512):
                        nc.tensor.matmul(out=hps[:, n * 512:(n + 1) * 512],
                                         lhsT=r_tile(g),
                                         rhs=wts[g][:, n * 512:(n + 1) * 512],
                                         start=True, stop=True)
                    hsb = hstp.tile([128, IN_F], BF, tag="hsb")
                    # Pool/GPSIMD cannot read PSUM on HW: DVE/Act only.
                    if g % 2 == 0:
                        nc.vector.tensor_copy(out=hsb[:], in_=hps[:])
                    else:
                        nc.scalar.copy(out=hsb[:], in_=hps[:])
                    nc.sync.dma_start(hnat_d[g * 128:(g + 1) * 128, :IN_F],
                                      hsb[:])

            with tc.tile_pool(name="sbc2", bufs=2) as sbc2, \
                 tc.tile_pool(name="psc2", bufs=4, space="PSUM") as psc2:
                drive([cayley_quad(q, sbc2, psc2) for q in (4, 5, 6, 7)])

        # --- Phase T+G: transpose-gather rows of H(+bias) by inv_perm_out,
        #     fused per-gc with G = BD(R_right) @ H2T chunk matmuls.
        with tc.tile_pool(name="sbg", bufs=3) as sbg, \
             tc.tile_pool(name="biasp", bufs=1) as biasp:
            load_xts = make_xts_loader(sbg)
            load_xts(0)
            load_xts(1)
            bias_sb = biasp.tile([128, OUT_F], dt.float32)

            with tc.tile_pool(name="h2tp", bufs=2) as h2tp, \
                 tc.tile_pool(name="psg", bufs=1, space="PSUM") as psg:
                for gc in range(NB // 4):
                    h2t = h2tp.tile([128, KB + 1, 512], BF, tag="h2t")
                    nc.gpsimd.dma_gather(
                        out_ap=h2t[:],
                        in_ap=hnat_d[:],
                        idxs_ap=C["gidx"][:, gc * 32:(gc + 1) * 32],
                        num_idxs=512, num_idxs_reg=512, elem_size=HCOL,
                        transpose=True, queue_num=1 + gc % 2)
                    for sub in range(2):
                        ss = slice(sub * 256, (sub + 1) * 256)
                        for pair in range(KB // 2):
                            pr = psg.tile([128, 2, 256], dt.float32,
                                          tag=f"gp{pair}")
                            for half in range(2):
                                i = pair * 2 + half
                                nc.tensor.matmul(out=pr[:, half, :],
                                                 lhsT=r_tile(16 + i),
                                                 rhs=h2t[:, i, ss],
                                                 start=True, stop=True)
                            col0 = gc * 512 + sub * 256
                            dst = C["weff"][:, 2 * pair:2 * pair + 2,
                                            col0:col0 + 256]
                            if pair % 2 == 0:
                                nc.vector.tensor_copy(out=dst, in_=pr[:])
                            else:
                                nc.scalar.copy(out=dst, in_=pr[:])
                    # permuted bias row for this gc (partition 0, chunk KB)
                    nc.vector.tensor_copy(
                        out=C["b2row"][0:1, gc * 512:(gc + 1) * 512],
                        in_=h2t[0:1, KB, :])

            # bias broadcast across partitions via K=1 ones-matmul
            with tc.tile_pool(name="psbias", bufs=1, space="PSUM") as psbias:
                bbps = psbias.tile([128, OUT_F], dt.float32)
                for n in range(OUT_F // 512):
                    nc.tensor.matmul(out=bbps[:, n * 512:(n + 1) * 512],
                                     lhsT=C["onesb"][:1, :],
                                     rhs=C["b2row"][:1, n * 512:(n + 1) * 512],
                                     start=True, stop=True)
                nc.vector.tensor_copy(out=bias_sb[:], in_=bbps[:])

            if debug:
                for k in range(KB):
                    nc.sync.dma_start(wdump_d[k * 128:(k + 1) * 128, :],
                                      C["weff"][:, k, :])
                nc.sync.dma_start(bdump_d[:], bias_sb[:])

            # ---------------- main GEMM ----------------
            with tc.tile_pool(name="osbp", bufs=2) as osbp, \
                 tc.tile_pool(name="psgm", bufs=2, space="PSUM") as psgm:
                for s in range(n_sup):
                    if s + 2 < n_sup:
                        load_xts(s + 2)
                    xts = xts_tiles.pop(s)
                    for mt in range(MT):
                        gps = psgm.tile([128, OUT_F], dt.float32, tag="gemmps")
                        for k in range(KB):
                            for n in range(OUT_F // 512):
                                nc.tensor.matmul(
                                    out=gps[:, n * 512:(n + 1) * 512],
                                    lhsT=xts[:, k, mt * 128:(mt + 1) * 128],
                                    rhs=C["weff"][:, k, n * 512:(n + 1) * 512],
                                    start=(k == 0), stop=(k == KB - 1))
                        osb = osbp.tile([128, OUT_F], dt.float32, tag="osb")
                        row0 = s * SUP + mt * 128
                        last = (s == n_sup - 1) and (mt == MT - 1)
                        if not last:
                            nc.vector.tensor_add(out=osb[:], in0=gps[:],
                                                 in1=bias_sb[:])
                            nc.sync.dma_start(out_d[row0:row0 + 128, :],
                                              osb[:])
                        else:
                            # chunk the final tile so add+store pipeline and
                            # the drain tail shrinks
                            for n in range(4):
                                sl = slice(n * 512, (n + 1) * 512)
                                nc.vector.tensor_add(out=osb[:, sl],
                                                     in0=gps[:, sl],
                                                     in1=bias_sb[:, sl])
                                nc.sync.dma_start(out_d[row0:row0 + 128, sl],
                                                  osb[:, sl])

    with tile.TileContext(nc) as tc:
        # Constants + long-lived tiles, emitted once (outside the HW loop).
        with tc.tile_pool(name="const", bufs=1) as const, \
             tc.tile_pool(name="wfp", bufs=1) as wfp, \
             tc.tile_pool(name="b2rp", bufs=1) as b2rp, \
             tc.tile_pool(name="rpool", bufs=8) as rpool:
            ident = const.tile([128, 128], dt.float32)
            make_identity(nc, ident)
            identq = const.tile([128, 4, 128], BF)
            for gg in range(4):
                nc.vector.tensor_copy(out=identq[:, gg, :], in_=ident[:])
            gidx = const.tile([128, 8 * NB], dt.int16)
            nc.sync.dma_start(gidx[:], gout_in[:])
            b_sb = const.tile([128, NB], BF)
            nc.sync.dma_start(
                b_sb[:], b_in[:].rearrange("(g p) one -> p (g one)", p=128))
            onesb = const.tile([1, 128], BF)
            nc.vector.memset(onesb[:], 1.0)
            C = {
                "identq": identq, "gidx": gidx, "b_sb": b_sb, "onesb": onesb,
                "weff": wfp.tile([128, KB, OUT_F], BF, name="weff_all"),
                "b2row": b2rp.tile([1, OUT_F], BF, name="b2row"),
                "rpool": rpool,
            }
            if n_reps > 1:
                # Whole-kernel hardware loop: executes the full computation
                # n_reps times per dispatch so per-iteration HW time can be
                # measured above the host-side dispatch latency.
                with tc.For_i(0, n_reps):
                    _emit_body(tc, C)
            else:
                _emit_body(tc, C)

    nc.compile()
    return nc


def _wrap_idx16(idx):
    """Pack N gather indices into dma_gather's wrapped layout: index j at
    [j % 16, j // 16], replicated across the 8 Q7 cores -> [128, N//16]."""
    n = len(idx)
    arr = np.zeros((16, n // 16), np.int16)
    j = np.arange(n)
    arr[j % 16, j // 16] = idx.astype(np.int16)
    return np.tile(arr, (8, 1))


def _host_prep(inputs):
    from ml_dtypes import bfloat16
    rows = np.asarray(inputs["rows"]).astype(np.int64)
    cols = np.asarray(inputs["cols"]).astype(np.int64)
    emat = np.zeros((N_ELEM, BS * BS), dtype=np.float32)
    e_idx = np.arange(N_ELEM)
    emat[e_idx, rows * BS + cols] = 1.0
    emat[e_idx, cols * BS + rows] = -1.0
    emat = emat.astype(bfloat16)
    oft = np.ascontiguousarray(
        np.concatenate([np.asarray(inputs["oft_L"], dtype=np.float32),
                        np.asarray(inputs["oft_R"], dtype=np.float32)],
                       axis=0).T).astype(bfloat16)  # host-transposed [496, 128]
    inv_pout = np.asarray(inputs["inv_perm_out"]).astype(np.int64)
    gout = np.concatenate([_wrap_idx16(inv_pout[gc * 256:(gc + 1) * 256])
                           for gc in range(NB // 2)], axis=1)
    w = np.ascontiguousarray(np.asarray(inputs["W"], dtype=np.float32)).astype(bfloat16)
    b = np.asarray(inputs["b"], dtype=np.float32).reshape(OUT_F, 1).astype(bfloat16)
    return emat, oft, gout, w, b


def _in_map(inputs):
    emat, oft, gout, w, b = _host_prep(inputs)
    return {"w": w, "b": b, "oft": oft, "emat": emat, "gout": gout}


def kernel(**inputs):
    from concourse.bass_utils import run_bass_kernel_spmd
    from ml_dtypes import bfloat16

    key = ("full", TOKPC)
    if key not in _CACHE:
        _CACHE[key] = _build(TOKPC)
    nc = _CACHE[key]

    x = np.asarray(inputs["x"], dtype=np.float32).reshape(TOKENS, IN_F)
    perm_in = np.asarray(inputs["perm_in"]).astype(np.int64)
    base = _in_map(inputs)
    in_maps = []
    for c in range(N_CORES):
        m = dict(base)
        # host-side layout: transpose the shard AND apply the input feature
        # permutation as a row gather (x @ P_in == P_in-rows of x^T)
        m["xt"] = np.ascontiguousarray(
            x[c * TOKPC:(c + 1) * TOKPC].T[perm_in]).astype(bfloat16)
        in_maps.append(m)

    res = run_bass_kernel_spmd(nc, in_maps, core_ids=list(range(N_CORES)))
    out = np.concatenate([res.results[c]["out"] for c in range(N_CORES)], axis=0)
    return out.reshape(4, 8192, OUT_F)


# revision 33
# speedup vs baseline: 1.0260x; 1.0175x over previous
"""TRN2 Bass kernel for nn_OFTLinear (forward).

Math: the whole OFT chain is linear, so
    out = x @ W_eff + b_eff
with
    W_eff = P_in . BD(R_right) . W^T . BD(R_left) . P_out      [2048 x 2048]
    b_eff = (BD(R_left)^T b)[inv_perm_out]
where R = Cayley-Neumann(skew(oft)) per 32x32 block, BD() is block-diagonal,
and P_in/P_out are the input/output feature permutations.

Device pipeline (replicated on all 8 cores; x sharded along tokens). The
whole W_eff build + GEMM runs in bf16 (fp32 PSUM accumulation): rel-err
budget is 2e-2 and bf16 lands ~4e-3. The Cayley series itself also runs in
bf16 -- its tiles are cast to bf16 for the matmuls anyway, so fp32
intermediates only added PE time (fp32 matmuls are 4x slower).

  Q:  Q_flat = vec^T @ E (E: host-built one-hot skew-scatter matrix),
      4-deep load buffers so the DMA/matmul ping-pong doesn't serialize.
  C:  BD4 tiles of Q (4 blocks per 128x128 tile) -> Cayley powers on PE ->
      R_left tiles (g<16) and R_right^T = R(-Q) tiles (g>=16), all bf16.
  B:  b_rot = BD_L^T b via 16 matvecs; stored as an EXTRA COLUMN (col 2048)
      of H in DRAM via one strided DMA, so the same row-gather that permutes
      W_eff also permutes the bias -- no indirect scatter chain.
  H:  H = BD_L^T @ W on PE, streamed to DRAM as [2048, 2176] bf16 rows
      (cols 0..2047 = H, col 2048 = bias, rest pad). W tiles are prefetched
      8 deep so the loads run during the Cayley phase.
  T+G: 8 dma_gather(transpose=True) ops (256 out-cols each) deliver
      H2T = gathered(H)^T into SBUF; each gather is immediately followed by
      its 16 G matmuls (G = BD(R_right) @ H2T chunk) into W_eff slices, so
      gathers and G pipeline per-gc instead of forming one serial wall.
  GEMM: W_eff k-tiles live in SBUF; out = xT.T@W_eff + bias. x supertiles
      are software-pipelined (prefetch depth 2) on the SP DMA queue, which
      is otherwise idle until output stores start.

Constants (identity, gather indices, bias layout) are built OUTSIDE the
n_reps hardware loop so they don't re-execute per iteration.

Host does layout-only work: shard x along tokens, transpose + bf16-cast each
shard (fp32 DMA transpose is unsupported on this stack), concat + bf16-cast
oft_L/oft_R, and build integer index/one-hot constants from the
permutation/index buffers.

n_reps > 1 wraps the computation in a tc.For_i hardware loop so one
dispatch executes the full kernel n_reps times back-to-back: per-iteration
HW time can then be measured above the host-side dispatch latency (which
dwarfs a single execution in this environment).
"""

import numpy as np

IN_F = 2048
OUT_F = 2048
BS = 32
N_ELEM = BS * (BS - 1) // 2  # 496
N_BLOCKS = 128  # 64 left + 64 right
N_CORES = 8
TOKENS = 4 * 8192
TOKPC = TOKENS // N_CORES  # 4096
KB = IN_F // 128  # 16 k-blocks
NB = OUT_F // 128  # 16 n-blocks
HCOL = IN_F + 128  # H DRAM row: 2048 cols + bias col + 127 pad (2176)

_CACHE = {}


def _build(tokpc, n_reps=1, debug=False):
    import concourse.bass as bass
    import concourse.bacc as bacc
    import concourse.mybir as mybir
    import concourse.tile as tile
    from concourse.masks import make_identity

    dt = mybir.dt
    BF = dt.bfloat16

    SUP = 256  # token super-tile
    n_sup = tokpc // SUP
    MT = SUP // 128  # m-tiles per super

    nc = bacc.Bacc(None, target_bir_lowering=False, debug=False,
                   enable_asserts=False, num_devices=1, num_swdge_queues=4)

    xt_in = nc.dram_tensor("xt", [IN_F, tokpc], BF, kind="ExternalInput").ap()
    w_in = nc.dram_tensor("w", [OUT_F, IN_F], BF, kind="ExternalInput").ap()
    b_in = nc.dram_tensor("b", [OUT_F, 1], BF, kind="ExternalInput").ap()
    # oft pre-transposed on host (layout-only): [N_ELEM, N_BLOCKS]
    oft_in = nc.dram_tensor("oft", [N_ELEM, N_BLOCKS], BF, kind="ExternalInput").ap()
    emat_in = nc.dram_tensor("emat", [N_ELEM, BS * BS], BF, kind="ExternalInput").ap()
    # inverse out-perm as wrapped int16 gather indices: [128, 8*16]
    gout_in = nc.dram_tensor("gout", [128, 8 * NB], dt.int16, kind="ExternalInput").ap()
    out_d = nc.dram_tensor("out", [tokpc, OUT_F], dt.float32, kind="ExternalOutput").ap()

    qflat_d = nc.dram_tensor("qflat_d", [N_BLOCKS, BS, BS], BF).ap()
    dbg_kw = {"kind": "ExternalOutput"} if debug else {}
    hnat_d = nc.dram_tensor("hnat_d", [OUT_F, HCOL], BF, **dbg_kw).ap()
    if debug:
        wdump_d = nc.dram_tensor("wdump", [IN_F, OUT_F], BF,
                                 kind="ExternalOutput").ap()
        bdump_d = nc.dram_tensor("bdump", [128, OUT_F], dt.float32,
                                 kind="ExternalOutput").ap()

    def _emit_body(tc, C):
        """One kernel execution. C holds the preloaded constants."""
        rq_by_q = {}

        def r_tile(g):
            return rq_by_q[g // 4][:, g % 4, :]

        xts_tiles = {}

        def make_xts_loader(sbg):
            xt_view = xt_in[:].rearrange("(k p) t -> p k t", p=128)

            def load_xts(s):
                t = sbg.tile([128, KB, SUP], BF, tag="xts")
                nc.sync.dma_start(t[:], xt_view[:, :, s * SUP:(s + 1) * SUP])
                xts_tiles[s] = t
            return load_xts

        # ---------------- Phase Q: Q_flat = vec^T @ E ----------------
        with tc.tile_pool(name="bdqp", bufs=1) as bdqp, \
             tc.tile_pool(name="wpool", bufs=16) as wpool:
            bdq_all = bdqp.tile([128, 32, 128], BF)
            nc.vector.memset(bdq_all[:], 0.0)
            with tc.tile_pool(name="sbq", bufs=1) as sbq, \
                 tc.tile_pool(name="psq", bufs=1, space="PSUM") as psq:
                qps = psq.tile([128, BS * BS], dt.float32)
                CH = 124  # 496 = 4 * 124 contraction chunks
                vt = sbq.tile([CH, 4, 128], BF)
                nc.scalar.dma_start(
                    vt[:], oft_in[:].rearrange("(c p) b -> p c b", c=4))
                et = sbq.tile([CH, 4, BS * BS], BF)
                nc.scalar.dma_start(
                    et[:], emat_in[:].rearrange("(c p) b -> p c b", c=4))
                for c in range(4):
                    for nh in range(2):
                        nc.tensor.matmul(out=qps[:, nh * 512:(nh + 1) * 512],
                                         lhsT=vt[:, c, :],
                                         rhs=et[:, c, nh * 512:(nh + 1) * 512],
                                         start=(c == 0), stop=(c == 3))
                # W loads up front: the scalar engine reaches these while the
                # PREVIOUS iteration's GEMM still owns the PE, so all of W is
                # in SBUF before this iteration's H phase starts.
                wts = []
                for g in range(NB):
                    wt = wpool.tile([128, IN_F], BF, tag="wt", name=f"wt_{g}")
                    nc.scalar.dma_start(wt[:],
                                        w_in[g * 128:(g + 1) * 128, :])
                    wts.append(wt)
                qsb = sbq.tile([128, BS * BS], BF)
                nc.vector.tensor_copy(out=qsb[:], in_=qps[:])
                nc.sync.dma_start(qflat_d[:].rearrange("p a b -> p (a b)"),
                                  qsb[:])

            # BD4 layout: quad q slot s holds blocks 4*(4q+s)..4*(4q+s)+3
            qview = qflat_d[:].rearrange("(g four) i j -> four i g j", four=4)
            for r in range(4):
                nc.sync.dma_start(
                    bdq_all[r * BS:(r + 1) * BS, :, r * BS:(r + 1) * BS],
                    qview[r])

            # ---------------- Phase C: Cayley powers (bf16) ----------------
            def cayley_quad(q, sbc, psc):
                """Generator: one DVE<->PE pipeline step per yield so quads
                interleave their latency chains. Sign-flipped intermediates
                fold the Neumann x2 scaling into the matmuls.
                   R(+Q) = I + (2Q^4 + 2Q^2) + (2Q + 2Q^3)   (q < 4)
                   R(-Q) = I + (2Q^4 + 2Q^2) - (2Q + 2Q^3)   (q >= 4)
                """
                ev = nc.vector
                sub = mybir.AluOpType.subtract
                bdq4 = bdq_all[:, 4 * q:4 * q + 4, :]
                s2q = sbc.tile([128, 4, 128], BF, tag="s2q")
                ev.tensor_scalar_mul(out=s2q[:], in0=bdq4, scalar1=2.0)
                yield
                m2p2ps = psc.tile([128, 4, 128], dt.float32, tag="p2ps")
                for gg in range(4):
                    nc.tensor.matmul(out=m2p2ps[:, gg, :], lhsT=bdq4[:, gg, :],
                                     rhs=s2q[:, gg, :], start=True, stop=True)
                m2p2 = sbc.tile([128, 4, 128], BF, tag="p2")
                nc.scalar.copy(out=m2p2[:], in_=m2p2ps[:])
                yield
                m2p3ps = psc.tile([128, 4, 128], dt.float32, tag="p3ps")
                for gg in range(4):
                    nc.tensor.matmul(out=m2p3ps[:, gg, :], lhsT=m2p2[:, gg, :],
                                     rhs=bdq4[:, gg, :], start=True, stop=True)
                m2p3 = sbc.tile([128, 4, 128], BF, tag="p3")
                nc.scalar.copy(out=m2p3[:], in_=m2p3ps[:])
                yield
                p4ps = psc.tile([128, 4, 128], dt.float32, tag="p2ps")
                for gg in range(4):
                    nc.tensor.matmul(out=p4ps[:, gg, :], lhsT=m2p3[:, gg, :],
                                     rhs=bdq4[:, gg, :], start=True, stop=True)
                t1 = sbc.tile([128, 4, 128], BF, tag="t1")
                ev.tensor_tensor(out=t1[:], in0=p4ps[:], in1=m2p2[:], op=sub)
                yield
                t2 = sbc.tile([128, 4, 128], BF, tag="t2")
                ev.tensor_tensor(out=t2[:], in0=s2q[:], in1=m2p3[:], op=sub)
                t3 = sbc.tile([128, 4, 128], BF, tag="t3")
                op = mybir.AluOpType.add if q < 4 else sub
                ev.tensor_tensor(out=t3[:], in0=t1[:], in1=t2[:], op=op)
                yield
                rq = C["rpool"].tile([128, 4, 128], BF, tag="rq",
                                     name=f"rq_{q}")
                ev.tensor_add(out=rq[:], in0=t3[:], in1=C["identq"][:])
                rq_by_q[q] = rq

            def drive(gens):
                gens = list(gens)
                while gens:
                    g = gens.pop(0)
                    try:
                        next(g)
                        gens.append(g)
                    except StopIteration:
                        pass

            with tc.tile_pool(name="sbc", bufs=2) as sbc, \
                 tc.tile_pool(name="psc", bufs=4, space="PSUM") as psc:
                drive([cayley_quad(q, sbc, psc) for q in (0, 1, 2, 3)])

            # Phase B: b_rot = BD_L^T b -> bias column (2048) of hnat_d
            # via one strided DMA store (bf16 R_left tiles; bias is tiny).
            with tc.tile_pool(name="sbb", bufs=1) as sbb, \
                 tc.tile_pool(name="psb", bufs=1, space="PSUM") as psb:
                brotps = psb.tile([128, NB], dt.float32)
                for g in range(NB):
                    nc.tensor.matmul(out=brotps[:, g:g + 1],
                                     lhsT=rq_by_q[g // 4][:, g % 4, :],
                                     rhs=C["b_sb"][:, g:g + 1],
                                     start=True, stop=True)
                brot = sbb.tile([128, NB], BF)
                nc.scalar.copy(out=brot[:], in_=brotps[:])
                nc.sync.dma_start(
                    hnat_d[:].rearrange("(g p) c -> p g c", p=128)[
                        :, :, IN_F:IN_F + 1].rearrange("p g one -> p (g one)"),
                    brot[:])

            # ------- Phase H: H = BD_L^T @ W -> DRAM rows, bf16 -------
            # Needs only quads 0-3; quads 4-7 (for G) are driven right
            # after so their DVE/PE latency chains hide under H's
            # DMA-gated execution.
            with tc.tile_pool(name="hstp", bufs=6) as hstp, \
                 tc.tile_pool(name="psh", bufs=2, space="PSUM") as psh:
                for g in range(NB):
                    hps = psh.tile([128, IN_F], dt.float32, tag="hps")
                    for n in range(IN_F // 512):
                        nc.tensor.matmul(out=hps[:, n * 512:(n + 1) * 512],
                                         lhsT=r_tile(g),
                                         rhs=wts[g][:, n * 512:(n + 1) * 512],
                                         start=True, stop=True)
                    hsb = hstp.tile([128, IN_F], BF, tag="hsb")
                    # Pool/GPSIMD cannot read PSUM on HW: DVE/Act only.
                    if g % 2 == 0:
                        nc.vector.tensor_copy(out=hsb[:], in_=hps[:])
                    else:
                        nc.scalar.copy(out=hsb[:], in_=hps[:])
                    nc.sync.dma_start(hnat_d[g * 128:(g + 1) * 128, :IN_F],
                                      hsb[:])

            with tc.tile_pool(name="sbc2", bufs=2) as sbc2, \
                 tc.tile_pool(name="psc2", bufs=4, space="PSUM") as psc2:
                drive([cayley_quad(q, sbc2, psc2) for q in (4, 5, 6, 7)])

        # --- Phase T+G: transpose-gather rows of H(+bias) by inv_perm_out,
        #     fused per-gc with G = BD(R_right) @ H2T chunk matmuls.
        with tc.tile_pool(name="sbg", bufs=3) as sbg, \
             tc.tile_pool(name="biasp", bufs=1) as biasp:
            load_xts = make_xts_loader(sbg)
            load_xts(0)
            load_xts(1)
            bias_sb = biasp.tile([128, OUT_F], dt.float32)

            with tc.tile_pool(name="h2tp", bufs=4) as h2tp, \
                 tc.tile_pool(name="psg", bufs=1, space="PSUM") as psg:
                h2ts = []
                for gc in range(NB // 4):
                    h2t = h2tp.tile([128, KB + 1, 512], BF, tag="h2t",
                                    name=f"h2t_{gc}")
                    nc.gpsimd.dma_gather(
                        out_ap=h2t[:],
                        in_ap=hnat_d[:],
                        idxs_ap=C["gidx"][:, gc * 32:(gc + 1) * 32],
                        num_idxs=512, num_idxs_reg=512, elem_size=HCOL,
                        transpose=True, queue_num=1 + gc % 2)
                    h2ts.append(h2t)
                for gc in range(NB // 4):
                    h2t = h2ts[gc]
                    for sub in range(2):
                        ss = slice(sub * 256, (sub + 1) * 256)
                        for pair in range(KB // 2):
                            pr = psg.tile([128, 2, 256], dt.float32,
                                          tag=f"gp{pair}")
                            for half in range(2):
                                i = pair * 2 + half
                                nc.tensor.matmul(out=pr[:, half, :],
                                                 lhsT=r_tile(16 + i),
                                                 rhs=h2t[:, i, ss],
                                                 start=True, stop=True)
                            col0 = gc * 512 + sub * 256
                            dst = C["weff"][:, 2 * pair:2 * pair + 2,
                                            col0:col0 + 256]
                            if pair % 2 == 0:
                                nc.vector.tensor_copy(out=dst, in_=pr[:])
                            else:
                                nc.scalar.copy(out=dst, in_=pr[:])
                    # permuted bias row for this gc (partition 0, chunk KB)
                    nc.vector.tensor_copy(
                        out=C["b2row"][0:1, gc * 512:(gc + 1) * 512],
                        in_=h2t[0:1, KB, :])

            # bias broadcast across partitions via K=1 ones-matmul
            with tc.tile_pool(name="psbias", bufs=1, space="PSUM") as psbias:
                bbps = psbias.tile([128, OUT_F], dt.float32)
                for n in range(OUT_F // 512):
                    nc.tensor.matmul(out=bbps[:, n * 512:(n + 1) * 512],
                                     lhsT=C["onesb"][:1, :],
                                     rhs=C["b2row"][:1, n * 512:(n + 1) * 512],
                                     start=True, stop=True)
                nc.vector.tensor_copy(out=bias_sb[:], in_=bbps[:])

            if debug:
                for k in range(KB):
                    nc.sync.dma_start(wdump_d[k * 128:(k + 1) * 128, :],
                                      C["weff"][:, k, :])
                nc.sync.dma_start(bdump_d[:], bias_sb[:])

            # ---------------- main GEMM ----------------
            with tc.tile_pool(name="osbp", bufs=2) as osbp, \
                 tc.tile_pool(name="psgm", bufs=2, space="PSUM") as psgm:
                for s in range(n_sup):
                    if s + 2 < n_sup:
                        load_xts(s + 2)
                    xts = xts_tiles.pop(s)
                    for mt in range(MT):
                        gps = psgm.tile([128, OUT_F], dt.float32, tag="gemmps")
                        for k in range(KB):
                            for n in range(OUT_F // 512):
                                nc.tensor.matmul(
                                    out=gps[:, n * 512:(n + 1) * 512],
                                    lhsT=xts[:, k, mt * 128:(mt + 1) * 128],
                                    rhs=C["weff"][:, k, n * 512:(n + 1) * 512],
                                    start=(k == 0), stop=(k == KB - 1))
                        osb = osbp.tile([128, OUT_F], dt.float32, tag="osb")
                        row0 = s * SUP + mt * 128
                        last = (s == n_sup - 1) and (mt == MT - 1)
                        if not last:
                            nc.vector.tensor_add(out=osb[:], in0=gps[:],
                                                 in1=bias_sb[:])
                            nc.sync.dma_start(out_d[row0:row0 + 128, :],
                                              osb[:])
                        else:
                            # chunk the final tile so add+store pipeline and
                            # the drain tail shrinks
                            for n in range(4):
                                sl = slice(n * 512, (n + 1) * 512)
                                nc.vector.tensor_add(out=osb[:, sl],
                                                     in0=gps[:, sl],
                                                     in1=bias_sb[:, sl])
                                nc.sync.dma_start(out_d[row0:row0 + 128, sl],
                                                  osb[:, sl])

    with tile.TileContext(nc) as tc:
        # Constants + long-lived tiles, emitted once (outside the HW loop).
        with tc.tile_pool(name="const", bufs=1) as const, \
             tc.tile_pool(name="wfp", bufs=1) as wfp, \
             tc.tile_pool(name="b2rp", bufs=1) as b2rp, \
             tc.tile_pool(name="rpool", bufs=8) as rpool:
            ident = const.tile([128, 128], dt.float32)
            make_identity(nc, ident)
            identq = const.tile([128, 4, 128], BF)
            for gg in range(4):
                nc.vector.tensor_copy(out=identq[:, gg, :], in_=ident[:])
            gidx = const.tile([128, 8 * NB], dt.int16)
            nc.sync.dma_start(gidx[:], gout_in[:])
            b_sb = const.tile([128, NB], BF)
            nc.sync.dma_start(
                b_sb[:], b_in[:].rearrange("(g p) one -> p (g one)", p=128))
            onesb = const.tile([1, 128], BF)
            nc.vector.memset(onesb[:], 1.0)
            C = {
                "identq": identq, "gidx": gidx, "b_sb": b_sb, "onesb": onesb,
                "weff": wfp.tile([128, KB, OUT_F], BF, name="weff_all"),
                "b2row": b2rp.tile([1, OUT_F], BF, name="b2row"),
                "rpool": rpool,
            }
            if n_reps > 1:
                # Whole-kernel hardware loop: executes the full computation
                # n_reps times per dispatch so per-iteration HW time can be
                # measured above the host-side dispatch latency.
                with tc.For_i(0, n_reps):
                    _emit_body(tc, C)
            else:
                _emit_body(tc, C)

    nc.compile()
    return nc


def _wrap_idx16(idx):
    """Pack N gather indices into dma_gather's wrapped layout: index j at
    [j % 16, j // 16], replicated across the 8 Q7 cores -> [128, N//16]."""
    n = len(idx)
    arr = np.zeros((16, n // 16), np.int16)
    j = np.arange(n)
    arr[j % 16, j // 16] = idx.astype(np.int16)
    return np.tile(arr, (8, 1))


def _host_prep(inputs):
    from ml_dtypes import bfloat16
    rows = np.asarray(inputs["rows"]).astype(np.int64)
    cols = np.asarray(inputs["cols"]).astype(np.int64)
    emat = np.zeros((N_ELEM, BS * BS), dtype=np.float32)
    e_idx = np.arange(N_ELEM)
    emat[e_idx, rows * BS + cols] = 1.0
    emat[e_idx, cols * BS + rows] = -1.0
    emat = emat.astype(bfloat16)
    oft = np.ascontiguousarray(
        np.concatenate([np.asarray(inputs["oft_L"], dtype=np.float32),
                        np.asarray(inputs["oft_R"], dtype=np.float32)],
                       axis=0).T).astype(bfloat16)  # host-transposed [496, 128]
    inv_pout = np.asarray(inputs["inv_perm_out"]).astype(np.int64)
    gout = np.concatenate([_wrap_idx16(inv_pout[gc * 256:(gc + 1) * 256])
                           for gc in range(NB // 2)], axis=1)
    w = np.ascontiguousarray(np.asarray(inputs["W"], dtype=np.float32)).astype(bfloat16)
    b = np.asarray(inputs["b"], dtype=np.float32).reshape(OUT_F, 1).astype(bfloat16)
    return emat, oft, gout, w, b


def _in_map(inputs):
    emat, oft, gout, w, b = _host_prep(inputs)
    return {"w": w, "b": b, "oft": oft, "emat": emat, "gout": gout}


def kernel(**inputs):
    from concourse.bass_utils import run_bass_kernel_spmd
    from ml_dtypes import bfloat16

    key = ("full", TOKPC)
    if key not in _CACHE:
        _CACHE[key] = _build(TOKPC)
    nc = _CACHE[key]

    x = np.asarray(inputs["x"], dtype=np.float32).reshape(TOKENS, IN_F)
    perm_in = np.asarray(inputs["perm_in"]).astype(np.int64)
    base = _in_map(inputs)
    in_maps = []
    for c in range(N_CORES):
        m = dict(base)
        # host-side layout: transpose the shard AND apply the input feature
        # permutation as a row gather (x @ P_in == P_in-rows of x^T)
        m["xt"] = np.ascontiguousarray(
            x[c * TOKPC:(c + 1) * TOKPC].T[perm_in]).astype(bfloat16)
        in_maps.append(m)

    res = run_bass_kernel_spmd(nc, in_maps, core_ids=list(range(N_CORES)))
    out = np.concatenate([res.results[c]["out"] for c in range(N_CORES)], axis=0)
    return out.reshape(4, 8192, OUT_F)


# revision 35
# speedup vs baseline: 1.0619x; 1.0350x over previous
"""TRN2 Bass kernel for nn_OFTLinear (forward).

Math: the whole OFT chain is linear, so
    out = x @ W_eff + b_eff
with
    W_eff = P_in . BD(R_right) . W^T . BD(R_left) . P_out      [2048 x 2048]
    b_eff = (BD(R_left)^T b)[inv_perm_out]
where R = Cayley-Neumann(skew(oft)) per 32x32 block, BD() is block-diagonal,
and P_in/P_out are the input/output feature permutations.

Device pipeline (replicated on all 8 cores; x sharded along tokens). The
whole W_eff build + GEMM runs in bf16 (fp32 PSUM accumulation): rel-err
budget is 2e-2 and bf16 lands ~4e-3. The Cayley series itself also runs in
bf16 -- its tiles are cast to bf16 for the matmuls anyway, so fp32
intermediates only added PE time (fp32 matmuls are 4x slower).

  Q:  Q_flat = vec^T @ E (E: host-built one-hot skew-scatter matrix),
      4-deep load buffers so the DMA/matmul ping-pong doesn't serialize.
  C:  BD4 tiles of Q (4 blocks per 128x128 tile) -> Cayley powers on PE ->
      R_left tiles (g<16) and R_right^T = R(-Q) tiles (g>=16), all bf16.
  B:  b_rot = BD_L^T b via 16 matvecs; stored as an EXTRA COLUMN (col 2048)
      of H in DRAM via one strided DMA, so the same row-gather that permutes
      W_eff also permutes the bias -- no indirect scatter chain.
  H:  H = BD_L^T @ W on PE, streamed to DRAM as [2048, 2176] bf16 rows
      (cols 0..2047 = H, col 2048 = bias, rest pad). W tiles are prefetched
      8 deep so the loads run during the Cayley phase.
  T+G: 8 dma_gather(transpose=True) ops (256 out-cols each) deliver
      H2T = gathered(H)^T into SBUF; each gather is immediately followed by
      its 16 G matmuls (G = BD(R_right) @ H2T chunk) into W_eff slices, so
      gathers and G pipeline per-gc instead of forming one serial wall.
  GEMM: W_eff k-tiles live in SBUF; out = xT.T@W_eff + bias. x supertiles
      are software-pipelined (prefetch depth 2) on the SP DMA queue, which
      is otherwise idle until output stores start.

Constants (identity, gather indices, bias layout) are built OUTSIDE the
n_reps hardware loop so they don't re-execute per iteration.

Host does layout-only work: shard x along tokens, transpose + bf16-cast each
shard (fp32 DMA transpose is unsupported on this stack), concat + bf16-cast
oft_L/oft_R, and build integer index/one-hot constants from the
permutation/index buffers.

n_reps > 1 wraps the computation in a tc.For_i hardware loop so one
dispatch executes the full kernel n_reps times back-to-back: per-iteration
HW time can then be measured above the host-side dispatch latency (which
dwarfs a single execution in this environment).
"""

import numpy as np

IN_F = 2048
OUT_F = 2048
BS = 32
N_ELEM = BS * (BS - 1) // 2  # 496
N_BLOCKS = 128  # 64 left + 64 right
N_CORES = 8
TOKENS = 4 * 8192
TOKPC = TOKENS // N_CORES  # 4096
KB = IN_F // 128  # 16 k-blocks
NB = OUT_F // 128  # 16 n-blocks
HCOL = IN_F + 128  # H DRAM row: 2048 cols + bias col + 127 pad (2176)

_CACHE = {}


def _build(tokpc, n_reps=1, debug=False):
    import concourse.bass as bass
    import concourse.bacc as bacc
    import concourse.mybir as mybir
    import concourse.tile as tile
    from concourse.masks import make_identity

    dt = mybir.dt
    BF = dt.bfloat16

    SUP = 256  # token super-tile
    n_sup = tokpc // SUP
    MT = SUP // 128  # m-tiles per super

    nc = bacc.Bacc(None, target_bir_lowering=False, debug=False,
                   enable_asserts=False, num_devices=1, num_swdge_queues=4)

    xt_in = nc.dram_tensor("xt", [IN_F, tokpc], BF, kind="ExternalInput").ap()
    w_in = nc.dram_tensor("w", [OUT_F, IN_F], BF, kind="ExternalInput").ap()
    b_in = nc.dram_tensor("b", [OUT_F, 1], BF, kind="ExternalInput").ap()
    # oft pre-transposed on host (layout-only): [N_ELEM, N_BLOCKS]
    oft_in = nc.dram_tensor("oft", [N_ELEM, N_BLOCKS], BF, kind="ExternalInput").ap()
    emat_in = nc.dram_tensor("emat", [N_ELEM, BS * BS], BF, kind="ExternalInput").ap()
    # inverse out-perm as wrapped int16 gather indices: [128, 8*16]
    gout_in = nc.dram_tensor("gout", [128, 8 * NB], dt.int16, kind="ExternalInput").ap()
    out_d = nc.dram_tensor("out", [tokpc, OUT_F], dt.float32, kind="ExternalOutput").ap()

    qflat_d = nc.dram_tensor("qflat_d", [N_BLOCKS, BS, BS], BF).ap()
    dbg_kw = {"kind": "ExternalOutput"} if debug else {}
    hnat_d = nc.dram_tensor("hnat_d", [OUT_F, HCOL], BF, **dbg_kw).ap()
    if debug:
        wdump_d = nc.dram_tensor("wdump", [IN_F, OUT_F], BF,
                                 kind="ExternalOutput").ap()
        bdump_d = nc.dram_tensor("bdump", [128, OUT_F], dt.float32,
                                 kind="ExternalOutput").ap()

    def _emit_body(tc, C):
        """One kernel execution. C holds the preloaded constants."""
        rq_by_q = {}

        def r_tile(g):
            return rq_by_q[g // 4][:, g % 4, :]

        xts_tiles = {}

        def make_xts_loader(sbg):
            xt_view = xt_in[:].rearrange("(k p) t -> p k t", p=128)

            def load_xts(s):
                t = sbg.tile([128, KB, SUP], BF, tag="xts")
                nc.sync.dma_start(t[:], xt_view[:, :, s * SUP:(s + 1) * SUP])
                xts_tiles[s] = t
            return load_xts

        # ---------------- Phase Q: Q_flat = vec^T @ E ----------------
        with tc.tile_pool(name="bdqp", bufs=1) as bdqp, \
             tc.tile_pool(name="wpool", bufs=16) as wpool:
            bdq_all = bdqp.tile([128, 32, 128], BF)
            nc.vector.memset(bdq_all[:], 0.0)
            with tc.tile_pool(name="sbq", bufs=4) as sbq, \
                 tc.tile_pool(name="psq", bufs=1, space="PSUM") as psq:
                qps = psq.tile([128, BS * BS], dt.float32)
                CH = 124
                for c in range(4):
                    lo = c * CH
                    sz = min(CH, N_ELEM - lo)
                    vt = sbq.tile([CH, 128], BF, tag="vt")
                    nc.scalar.dma_start(vt[:sz, :], oft_in[lo:lo + sz, :])
                    et = sbq.tile([CH, BS * BS], BF, tag="et")
                    nc.scalar.dma_start(et[:sz, :], emat_in[lo:lo + sz, :])
                    for nh in range(2):
                        nc.tensor.matmul(out=qps[:, nh * 512:(nh + 1) * 512],
                                         lhsT=vt[:sz, :],
                                         rhs=et[:sz, nh * 512:(nh + 1) * 512],
                                         start=(c == 0), stop=(c == 3))
                # W loads up front: the scalar engine reaches these while the
                # PREVIOUS iteration's GEMM still owns the PE, so all of W is
                # in SBUF before this iteration's H phase starts.
                wts = []
                for g in range(NB):
                    wt = wpool.tile([128, IN_F], BF, tag="wt", name=f"wt_{g}")
                    nc.scalar.dma_start(wt[:],
                                        w_in[g * 128:(g + 1) * 128, :])
                    wts.append(wt)
                qsb = sbq.tile([128, BS * BS], BF)
                nc.vector.tensor_copy(out=qsb[:], in_=qps[:])
                nc.sync.dma_start(qflat_d[:].rearrange("p a b -> p (a b)"),
                                  qsb[:])

            # BD4 layout: quad q slot s holds blocks 4*(4q+s)..4*(4q+s)+3
            qview = qflat_d[:].rearrange("(g four) i j -> four i g j", four=4)
            for r in range(4):
                nc.sync.dma_start(
                    bdq_all[r * BS:(r + 1) * BS, :, r * BS:(r + 1) * BS],
                    qview[r])

            # ---------------- Phase C: Cayley powers (bf16) ----------------
            def cayley_quad(q, sbc, psc):
                """Generator: one DVE<->PE pipeline step per yield so quads
                interleave their latency chains. Sign-flipped intermediates
                fold the Neumann x2 scaling into the matmuls.
                   R(+Q) = I + (2Q^4 + 2Q^2) + (2Q + 2Q^3)   (q < 4)
                   R(-Q) = I + (2Q^4 + 2Q^2) - (2Q + 2Q^3)   (q >= 4)
                """
                ev = nc.vector
                sub = mybir.AluOpType.subtract
                bdq4 = bdq_all[:, 4 * q:4 * q + 4, :]
                s2q = sbc.tile([128, 4, 128], BF, tag="s2q")
                ev.tensor_scalar_mul(out=s2q[:], in0=bdq4, scalar1=2.0)
                yield
                m2p2ps = psc.tile([128, 4, 128], dt.float32, tag="p2ps")
                for gg in range(4):
                    nc.tensor.matmul(out=m2p2ps[:, gg, :], lhsT=bdq4[:, gg, :],
                                     rhs=s2q[:, gg, :], start=True, stop=True)
                m2p2 = sbc.tile([128, 4, 128], BF, tag="p2")
                nc.scalar.copy(out=m2p2[:], in_=m2p2ps[:])
                yield
                m2p3ps = psc.tile([128, 4, 128], dt.float32, tag="p3ps")
                for gg in range(4):
                    nc.tensor.matmul(out=m2p3ps[:, gg, :], lhsT=m2p2[:, gg, :],
                                     rhs=bdq4[:, gg, :], start=True, stop=True)
                m2p3 = sbc.tile([128, 4, 128], BF, tag="p3")
                nc.scalar.copy(out=m2p3[:], in_=m2p3ps[:])
                yield
                p4ps = psc.tile([128, 4, 128], dt.float32, tag="p2ps")
                for gg in range(4):
                    nc.tensor.matmul(out=p4ps[:, gg, :], lhsT=m2p3[:, gg, :],
                                     rhs=bdq4[:, gg, :], start=True, stop=True)
                t1 = sbc.tile([128, 4, 128], BF, tag="t1")
                ev.tensor_tensor(out=t1[:], in0=p4ps[:], in1=m2p2[:], op=sub)
                yield
                t2 = sbc.tile([128, 4, 128], BF, tag="t2")
                ev.tensor_tensor(out=t2[:], in0=s2q[:], in1=m2p3[:], op=sub)
                t3 = sbc.tile([128, 4, 128], BF, tag="t3")
                op = mybir.AluOpType.add if q < 4 else sub
                ev.tensor_tensor(out=t3[:], in0=t1[:], in1=t2[:], op=op)
                yield
                rq = C["rpool"].tile([128, 4, 128], BF, tag="rq",
                                     name=f"rq_{q}")
                ev.tensor_add(out=rq[:], in0=t3[:], in1=C["identq"][:])
                rq_by_q[q] = rq

            def drive(gens):
                gens = list(gens)
                while gens:
                    g = gens.pop(0)
                    try:
                        next(g)
                        gens.append(g)
                    except StopIteration:
                        pass

            with tc.tile_pool(name="sbc", bufs=2) as sbc, \
                 tc.tile_pool(name="psc", bufs=4, space="PSUM") as psc:
                drive([cayley_quad(q, sbc, psc) for q in (0, 1, 2, 3)])

            # Phase B: b_rot = BD_L^T b -> bias column (2048) of hnat_d
            # via one strided DMA store (bf16 R_left tiles; bias is tiny).
            with tc.tile_pool(name="sbb", bufs=1) as sbb, \
                 tc.tile_pool(name="psb", bufs=1, space="PSUM") as psb:
                brotps = psb.tile([128, NB], dt.float32)
                for g in range(NB):
                    nc.tensor.matmul(out=brotps[:, g:g + 1],
                                     lhsT=rq_by_q[g // 4][:, g % 4, :],
                                     rhs=C["b_sb"][:, g:g + 1],
                                     start=True, stop=True)
                brot = sbb.tile([128, NB], BF)
                nc.scalar.copy(out=brot[:], in_=brotps[:])
                nc.sync.dma_start(
                    hnat_d[:].rearrange("(g p) c -> p g c", p=128)[
                        :, :, IN_F:IN_F + 1].rearrange("p g one -> p (g one)"),
                    brot[:])

            # ------- Phase H: H = BD_L^T @ W -> DRAM rows, bf16 -------
            # Needs only quads 0-3; quads 4-7 (for G) are driven right
            # after so their DVE/PE latency chains hide under H's
            # DMA-gated execution.
            with tc.tile_pool(name="hstp", bufs=6) as hstp, \
                 tc.tile_pool(name="psh", bufs=2, space="PSUM") as psh:
                for g in range(NB):
                    hps = psh.tile([128, IN_F], dt.float32, tag="hps")
                    for n in range(IN_F // 512):
                        nc.tensor.matmul(out=hps[:, n * 512:(n + 1) * 512],
                                         lhsT=r_tile(g),
                                         rhs=wts[g][:, n * 512:(n + 1) * 512],
                                         start=True, stop=True)
                    hsb = hstp.tile([128, IN_F], BF, tag="hsb")
                    # Pool/GPSIMD cannot read PSUM on HW: DVE/Act only.
                    if g % 2 == 0:
                        nc.vector.tensor_copy(out=hsb[:], in_=hps[:])
                    else:
                        nc.scalar.copy(out=hsb[:], in_=hps[:])
                    nc.sync.dma_start(hnat_d[g * 128:(g + 1) * 128, :IN_F],
                                      hsb[:])

            with tc.tile_pool(name="sbc2", bufs=2) as sbc2, \
                 tc.tile_pool(name="psc2", bufs=4, space="PSUM") as psc2:
                drive([cayley_quad(q, sbc2, psc2) for q in (4, 5, 6, 7)])

        # --- Phase T+G: transpose-gather rows of H(+bias) by inv_perm_out,
        #     fused per-gc with G = BD(R_right) @ H2T chunk matmuls.
        with tc.tile_pool(name="sbg", bufs=3) as sbg, \
             tc.tile_pool(name="biasp", bufs=1) as biasp:
            load_xts = make_xts_loader(sbg)
            load_xts(0)
            load_xts(1)
            bias_sb = biasp.tile([128, OUT_F], dt.float32)

            with tc.tile_pool(name="h2tp", bufs=2) as h2tp, \
                 tc.tile_pool(name="psg", bufs=1, space="PSUM") as psg:
                for gc in range(NB // 4):
                    h2t = h2tp.tile([128, KB + 1, 512], BF, tag="h2t")
                    nc.gpsimd.dma_gather(
                        out_ap=h2t[:],
                        in_ap=hnat_d[:],
                        idxs_ap=C["gidx"][:, gc * 32:(gc + 1) * 32],
                        num_idxs=512, num_idxs_reg=512, elem_size=HCOL,
                        transpose=True, queue_num=1 + gc % 2)
                    for sub in range(2):
                        ss = slice(sub * 256, (sub + 1) * 256)
                        for pair in range(KB // 2):
                            pr = psg.tile([128, 2, 256], dt.float32,
                                          tag=f"gp{pair}")
                            for half in range(2):
                                i = pair * 2 + half
                                nc.tensor.matmul(out=pr[:, half, :],
                                                 lhsT=r_tile(16 + i),
                                                 rhs=h2t[:, i, ss],
                                                 start=True, stop=True)
                            col0 = gc * 512 + sub * 256
                            dst = C["weff"][:, 2 * pair:2 * pair + 2,
                                            col0:col0 + 256]
                            if pair % 2 == 0:
                                nc.vector.tensor_copy(out=dst, in_=pr[:])
                            else:
                                nc.scalar.copy(out=dst, in_=pr[:])
                    # permuted bias row for this gc (partition 0, chunk KB)
                    nc.vector.tensor_copy(
                        out=C["b2row"][0:1, gc * 512:(gc + 1) * 512],
                        in_=h2t[0:1, KB, :])

            # bias broadcast across partitions via K=1 ones-matmul
            with tc.tile_pool(name="psbias", bufs=1, space="PSUM") as psbias:
                bbps = psbias.tile([128, OUT_F], dt.float32)
                for n in range(OUT_F // 512):
                    nc.tensor.matmul(out=bbps[:, n * 512:(n + 1) * 512],
                                     lhsT=C["onesb"][:1, :],
                                     rhs=C["b2row"][:1, n * 512:(n + 1) * 512],
                                     start=True, stop=True)
                nc.vector.tensor_copy(out=bias_sb[:], in_=bbps[:])

            if debug:
                for k in range(KB):
                    nc.sync.dma_start(wdump_d[k * 128:(k + 1) * 128, :],
                                      C["weff"][:, k, :])
                nc.sync.dma_start(bdump_d[:], bias_sb[:])

            # ---------------- main GEMM ----------------
            with tc.tile_pool(name="osbp", bufs=2) as osbp, \
                 tc.tile_pool(name="psgm", bufs=2, space="PSUM") as psgm:
                for s in range(n_sup):
                    if s + 2 < n_sup:
                        load_xts(s + 2)
                    xts = xts_tiles.pop(s)
                    for mt in range(MT):
                        gps = psgm.tile([128, OUT_F], dt.float32, tag="gemmps")
                        for k in range(KB):
                            for n in range(OUT_F // 512):
                                nc.tensor.matmul(
                                    out=gps[:, n * 512:(n + 1) * 512],
                                    lhsT=xts[:, k, mt * 128:(mt + 1) * 128],
                                    rhs=C["weff"][:, k, n * 512:(n + 1) * 512],
                                    start=(k == 0), stop=(k == KB - 1))
                        osb = osbp.tile([128, OUT_F], dt.float32, tag="osb")
                        row0 = s * SUP + mt * 128
                        last = (s == n_sup - 1) and (mt == MT - 1)
                        if not last:
                            nc.vector.tensor_add(out=osb[:], in0=gps[:],
                                                 in1=bias_sb[:])
                            nc.sync.dma_start(out_d[row0:row0 + 128, :],
                                              osb[:])
                        else:
                            # chunk the final tile so add+store pipeline and
                            # the drain tail shrinks
                            for n in range(4):
                                sl = slice(n * 512, (n + 1) * 512)
                                nc.vector.tensor_add(out=osb[:, sl],
                                                     in0=gps[:, sl],
                                                     in1=bias_sb[:, sl])
                                nc.sync.dma_start(out_d[row0:row0 + 128, sl],
                                                  osb[:, sl])

    with tile.TileContext(nc) as tc:
        # Constants + long-lived tiles, emitted once (outside the HW loop).
        with tc.tile_pool(name="const", bufs=1) as const, \
             tc.tile_pool(name="wfp", bufs=1) as wfp, \
             tc.tile_pool(name="b2rp", bufs=1) as b2rp, \
             tc.tile_pool(name="rpool", bufs=8) as rpool:
            ident = const.tile([128, 128], dt.float32)
            make_identity(nc, ident)
            identq = const.tile([128, 4, 128], BF)
            for gg in range(4):
                nc.vector.tensor_copy(out=identq[:, gg, :], in_=ident[:])
            gidx = const.tile([128, 8 * NB], dt.int16)
            nc.sync.dma_start(gidx[:], gout_in[:])
            b_sb = const.tile([128, NB], BF)
            nc.sync.dma_start(
                b_sb[:], b_in[:].rearrange("(g p) one -> p (g one)", p=128))
            onesb = const.tile([1, 128], BF)
            nc.vector.memset(onesb[:], 1.0)
            C = {
                "identq": identq, "gidx": gidx, "b_sb": b_sb, "onesb": onesb,
                "weff": wfp.tile([128, KB, OUT_F], BF, name="weff_all"),
                "b2row": b2rp.tile([1, OUT_F], BF, name="b2row"),
                "rpool": rpool,
            }
            if n_reps > 1:
                # Whole-kernel hardware loop: executes the full computation
                # n_reps times per dispatch so per-iteration HW time can be
                # measured above the host-side dispatch latency.
                with tc.For_i(0, n_reps):
                    _emit_body(tc, C)
            else:
                _emit_body(tc, C)

    nc.compile()
    return nc


def _wrap_idx16(idx):
    """Pack N gather indices into dma_gather's wrapped layout: index j at
    [j % 16, j // 16], replicated across the 8 Q7 cores -> [128, N//16]."""
    n = len(idx)
    arr = np.zeros((16, n // 16), np.int16)
    j = np.arange(n)
    arr[j % 16, j // 16] = idx.astype(np.int16)
    return np.tile(arr, (8, 1))


def _host_prep(inputs):
    from ml_dtypes import bfloat16
    rows = np.asarray(inputs["rows"]).astype(np.int64)
    cols = np.asarray(inputs["cols"]).astype(np.int64)
    emat = np.zeros((N_ELEM, BS * BS), dtype=np.float32)
    e_idx = np.arange(N_ELEM)
    emat[e_idx, rows * BS + cols] = 1.0
    emat[e_idx, cols * BS + rows] = -1.0
    emat = emat.astype(bfloat16)
    oft = np.ascontiguousarray(
        np.concatenate([np.asarray(inputs["oft_L"], dtype=np.float32),
                        np.asarray(inputs["oft_R"], dtype=np.float32)],
                       axis=0).T).astype(bfloat16)  # host-transposed [496, 128]
    inv_pout = np.asarray(inputs["inv_perm_out"]).astype(np.int64)
    gout = np.concatenate([_wrap_idx16(inv_pout[gc * 256:(gc + 1) * 256])
                           for gc in range(NB // 2)], axis=1)
    w = np.ascontiguousarray(np.asarray(inputs["W"], dtype=np.float32)).astype(bfloat16)
    b = np.asarray(inputs["b"], dtype=np.float32).reshape(OUT_F, 1).astype(bfloat16)
    return emat, oft, gout, w, b


def _in_map(inputs):
    emat, oft, gout, w, b = _host_prep(inputs)
    return {"w": w, "b": b, "oft": oft, "emat": emat, "gout": gout}


def kernel(**inputs):
    from concourse.bass_utils import run_bass_kernel_spmd
    from ml_dtypes import bfloat16

    key = ("full", TOKPC)
    if key not in _CACHE:
        _CACHE[key] = _build(TOKPC)
    nc = _CACHE[key]

    x = np.asarray(inputs["x"], dtype=np.float32).reshape(TOKENS, IN_F)
    perm_in = np.asarray(inputs["perm_in"]).astype(np.int64)
    base = _in_map(inputs)
    in_maps = []
    for c in range(N_CORES):
        m = dict(base)
        # host-side layout: transpose the shard AND apply the input feature
        # permutation as a row gather (x @ P_in == P_in-rows of x^T)
        m["xt"] = np.ascontiguousarray(
            x[c * TOKPC:(c + 1) * TOKPC].T[perm_in]).astype(bfloat16)
        in_maps.append(m)

    res = run_bass_kernel_spmd(nc, in_maps, core_ids=list(range(N_CORES)))
    out = np.concatenate([res.results[c]["out"] for c in range(N_CORES)], axis=0)
    return out.reshape(4, 8192, OUT_F)
